# revision 11
# baseline (speedup 1.0000x reference)
"""Dynamic-expert-conv kernel for Trainium2 (8 NeuronCores, SPMD data-parallel).

Problem: per-sample expert-mixed 3x3 conv:
    w[b] = sum_e attention[b,e] * weights[e]     # [O, C, 3, 3]
    out[b] = conv2d(input[b], w[b], pad=1) + bias_mix[b][:, None, None]

Strategy (per core, 4 samples):
  - Expert weight bank resident in SBUF as bf16 (halves HBM+SBUF cost).
  - Per-sample combined weights built as a bf16 product/add TREE split
    across DVE and Pool: products via tensor_scalar_mul (4x DVE mode,
    ~0.31 ns/col) and pairwise tensor_tensor adds (2x mode, ~0.57),
    instead of the 1x scalar_tensor_tensor chain (~1.08). Combine drops
    from ~41us to ~10us/engine per sample, so it never gates the PE.
  - Conv as accumulating bf16 PE matmuls with the STATIONARY operand
    maximally reused: ONE sweep per (sample, o-chunk) over all 7 y-tile
    PSUM banks, (c-chunk, offset) outer / y-tiles inner. 18 stationary
    loads per sweep feed 7x448 moving columns each (144 LDWEIGHTS total
    after dedup, vs 288 for split groups).
  - PSUM evacuation (bias fused) round-robins over ScalarE (activation),
    DVE and Pool (tensor_scalar_add) so banks free at ~3x one engine's
    rate; the 8th PSUM bank is a rotation spare so the next sweep's
    first matmul never waits.
  - DMA rings: sync = att/bias/bank + all output stores; scalar = half
    the bank + per-sample input loads (issued between evac groups);
    gpsimd = sample-0 input only (Pool is combining the rest of the
    time). Input is bf16, host pre-padded to 58x58.
"""
import numpy as np

import concourse.bass as bass
import concourse.tile as tile
from concourse import bacc, mybir
from concourse.bass import ts
from concourse.bass_utils import run_bass_kernel_spmd
from contextlib import ExitStack

F32 = mybir.dt.float32
BF16 = mybir.dt.bfloat16
ADD = mybir.AluOpType.add
MULT = mybir.AluOpType.mult

B, C, O, H, W, KK, E = 32, 256, 256, 56, 56, 3, 8
N_CORES = 8
B_LOC = B // N_CORES          # 4 samples per core
PW = H + 2                    # 58 padded
CCH = C // 128                # 2
OCH = O // 128                # 2
YT = 8                        # output rows per tile
NT = H // YT                  # 7 y-tiles
DYX = KK * KK                 # 9
QH = DYX * 128                # 1152 combined-weight cols per (c-chunk, o-chunk)


def _dedup_ldweights(nc):
    """Drop redundant PE weight reloads.

    Tile lowering splits every bf16 matmul into Ldweights+Matmult; the HW
    pays a serial ~54ns per LDWEIGHTS and does not skip reloads of the
    already-loaded stationary. conv_sweep orders matmuls so consecutive
    ones share the stationary, so any Ldweights whose access pattern
    matches the previous Ldweights in the same block's PE stream (with no
    waits/updates of its own) is a no-op and can be deleted: the weights
    are still in the array, and its (empty) sync carries nothing."""
    removed = 0
    for blk in nc.m.functions[0].blocks:
        last_key = None
        keep = []
        for inst in blk.instructions:
            if isinstance(inst, mybir.InstLdweights):
                si = inst.sync_info
                clean = si is None or (not si.on_wait and not si.on_update)
                key = (str(inst.ins[0]), str(inst.tile_position),
                       str(inst.perf_mode), str(inst.is_transpose))
                if clean and key == last_key:
                    removed += 1
                    continue
                last_key = key
            keep.append(inst)
        if removed:
            blk.instructions = keep
    return removed


def build(iters: int = 1):
    nc = bacc.Bacc("TRN2", target_bir_lowering=False, debug=False,
                   num_devices=N_CORES)
    x = nc.dram_tensor("x", [B_LOC, 128, CCH, PW, PW], BF16,
                       kind="ExternalInput").ap()
    bank = nc.dram_tensor("bank", [E, 128, CCH, OCH, QH], BF16,
                          kind="ExternalInput").ap()
    att = nc.dram_tensor("att", [128, B_LOC * E], F32,
                         kind="ExternalInput").ap()
    bias_t = nc.dram_tensor("bias_t", [128, OCH, E], F32,
                            kind="ExternalInput").ap()
    out = nc.dram_tensor("out", [B_LOC, 128, OCH, H, W], F32,
                         kind="ExternalOutput").ap()

    with ExitStack() as ctx:
        tc = ctx.enter_context(tile.TileContext(nc))
        const = ctx.enter_context(tc.tile_pool(name="const", bufs=1))
        bankp = ctx.enter_context(tc.tile_pool(name="bankp", bufs=1))
        combp = ctx.enter_context(tc.tile_pool(name="combp", bufs=2))
        tmpp = ctx.enter_context(tc.tile_pool(name="tmpp", bufs=16))
        sampp = ctx.enter_context(tc.tile_pool(name="sampp", bufs=2))
        stagep = ctx.enter_context(tc.tile_pool(name="stagep", bufs=8))
        psump = ctx.enter_context(tc.tile_pool(name="psump", bufs=8,
                                               space="PSUM"))

        # att/bias first on the sync ring (tiny); the scalar ring opens
        # with the act-table load for the evacuation activations.
        att_sb = const.tile([128, B_LOC * E], F32)
        nc.sync.dma_start(att_sb[:], att[:])
        bias_sb = const.tile([128, OCH, E], F32)
        nc.sync.dma_start(bias_sb[:], bias_t[:])

        bank_sb = bankp.tile([128, E, CCH, OCH, QH], BF16)
        # Stream the bank in the (j,k) order combine consumes it, with
        # experts split across the sync and scalar rings so each quarter
        # lands ~2x sooner. The j=0 quarters (PE-start critical path) are
        # further split into the 3 column slices the chunked prologue
        # combine consumes, slice-major, so chunk 0's inputs all land
        # ~2.5us earlier than whole pieces would.
        for (j, k) in ((0, 0), (0, 1), (1, 0), (1, 1)):
            slices = ((0, 128), (128, 640), (640, QH)) if j == 0 \
                else ((0, QH),)
            for lo, hi in slices:
                for e in range(E):
                    ring = nc.sync if e % 2 == 0 else nc.scalar
                    ring.dma_start(bank_sb[:, e, k, j, lo:hi],
                                   bank[e][:, k, j, lo:hi])

        bias_comb = const.tile([128, B_LOC, OCH], F32)
        bias_junk = const.tile([128, E], F32)

        def combine_bias(b):
            # bias_comb[p, b, j] = sum_e bias_t[p, j, e] * att[p, b*E+e]
            for j in range(OCH):
                nc.vector.scalar_tensor_tensor(
                    bias_junk[:], bias_sb[:, j, :], 1.0,
                    att_sb[:, b * E:(b + 1) * E],
                    op0=MULT, op1=MULT,
                    accum_out=bias_comb[:, b, j:j + 1])

        def combine_quarter(b, cb, j, k, chunks=1, pool_share=True):
            """cb[:,k,j,:] = sum_e att[b,e]*bank[:,e,k,j,:], all in bf16.

            Product/add tree: products via tensor_scalar_mul (DVE 4x
            mode), pairwise adds via tensor_tensor (DVE 2x). With
            pool_share, experts 6,7 and the upper merges run on Pool
            (~balanced: DVE 6 mul + 4 add, Pool 2 mul + 3 add).
            `chunks` splits the columns so the first slice is ready for
            the PE sooner (prologue only)."""
            a = [att_sb[:, b * E + e:b * E + e + 1] for e in range(E)]
            t = [tmpp.tile([128, QH], BF16, name="tmp") for _ in range(8)]
            V = nc.vector
            P = nc.gpsimd if pool_share else nc.vector
            if chunks == 1:
                bounds = ((0, QH),)
            else:
                # Tiny first chunk so the PE's first matmul (which only
                # needs cols 0:128) starts as early as possible.
                bounds = ((0, 128), (128, 640), (640, QH))
            for lo_c, hi_c in bounds:
                s = slice(lo_c, hi_c)
                bk = [bank_sb[:, e, k, j, s] for e in range(E)]
                V.tensor_scalar_mul(t[0][:, s], bk[0], a[0])
                V.tensor_scalar_mul(t[1][:, s], bk[1], a[1])
                V.tensor_tensor(t[0][:, s], t[0][:, s], t[1][:, s], op=ADD)
                V.tensor_scalar_mul(t[2][:, s], bk[2], a[2])
                V.tensor_scalar_mul(t[3][:, s], bk[3], a[3])
                V.tensor_tensor(t[2][:, s], t[2][:, s], t[3][:, s], op=ADD)
                V.tensor_tensor(t[0][:, s], t[0][:, s], t[2][:, s], op=ADD)
                V.tensor_scalar_mul(t[4][:, s], bk[4], a[4])
                V.tensor_scalar_mul(t[5][:, s], bk[5], a[5])
                V.tensor_tensor(t[4][:, s], t[4][:, s], t[5][:, s], op=ADD)
                P.tensor_scalar_mul(t[6][:, s], bk[6], a[6])
                P.tensor_scalar_mul(t[7][:, s], bk[7], a[7])
                P.tensor_tensor(t[6][:, s], t[6][:, s], t[7][:, s], op=ADD)
                P.tensor_tensor(t[4][:, s], t[4][:, s], t[6][:, s], op=ADD)
                P.tensor_tensor(cb[:, k, j, s], t[0][:, s], t[4][:, s],
                                op=ADD)

        def combine_sample(b, cb, prologue=False):
            for j in range(OCH):
                for k in range(CCH):
                    if prologue and j == 0:
                        # Fine-grained, DVE-only (Pool busy with the
                        # sample-0 input DMA until ~5.5us).
                        combine_quarter(b, cb, j, k, chunks=3,
                                        pool_share=False)
                    else:
                        combine_quarter(b, cb, j, k)
                if j == 0:
                    combine_bias(b)

        def load_samp(b, ring):
            samp = sampp.tile([128, CCH, PW, PW], BF16, name="samp")
            ring.dma_start(samp[:], x[b][:])
            return samp

        def conv_sweep(j, comb, samp):
            """One stationary-load sweep per (sample, o-chunk): (k,d)
            outer so consecutive matmuls share one combined-weight tile
            (_dedup_ldweights drops their redundant reloads) and stream
            7x448 moving columns per load; all 7 y-tiles accumulate in
            parallel PSUM banks."""
            psums = [psump.tile([128, YT, W], F32, name="psum")
                     for _ in range(NT)]
            for k in range(CCH):
                for d in range(DYX):
                    dy, dx = d // KK, d % KK
                    lhsT = comb[:, k, j, d * 128:(d + 1) * 128]
                    first = (k == 0 and d == 0)
                    last = (k == CCH - 1 and d == DYX - 1)
                    for i in range(NT):
                        r0 = i * YT + dy
                        rhs = samp[:, k, r0:r0 + YT, dx:dx + W]
                        nc.tensor.matmul(psums[i][:], lhsT, rhs,
                                         start=first, stop=last)
            return psums

        def evac_sweep(b, j, psums, last=False):
            """PSUM -> SBUF fp32 with mixed bias fused, round-robin over
            ScalarE / DVE / Pool so banks free ~3x faster; store on the
            sync ring (the final sweep spreads stores over all 3 rings
            to cut the drain tail ~3x)."""
            stages = []
            for i in range(NT):
                stage = stagep.tile([128, YT, W], F32, name="stage")
                # Only ScalarE and DVE may read PSUM (GPSIMD is rejected
                # by the BIR verifier); alternate them so banks free at
                # 2x one engine's rate.
                if i % 2 == 0:
                    nc.scalar.activation(
                        stage[:], psums[i][:],
                        mybir.ActivationFunctionType.Identity,
                        bias=bias_comb[:, b, j:j + 1], scale=1.0)
                else:
                    nc.vector.tensor_scalar_add(stage[:], psums[i][:],
                                                bias_comb[:, b, j:j + 1])
                stages.append(stage)
            rings = ((nc.sync, nc.scalar, nc.gpsimd) if last
                     else (nc.sync,))
            for i in range(NT):
                rings[i % len(rings)].dma_start(
                    out[b][:, j:j + 1, ts(i, YT), :], stages[i][:])

        def body():
            samps = {0: load_samp(0, nc.gpsimd)}
            combs = {0: combp.tile([128, CCH, OCH, QH], BF16, name="comb")}
            combine_sample(0, combs[0], prologue=True)
            for b in range(B_LOC):
                for j in range(OCH):
                    psums = conv_sweep(j, combs[b], samps[b])
                    evac_sweep(b, j, psums,
                               last=(b == B_LOC - 1 and j == OCH - 1))
                    if j == 0 and b + 1 < B_LOC:
                        # Issue next sample's input + combine after the
                        # evac ops so PSUM turnaround never queues behind
                        # them; they execute during this sample's j=1.
                        samps[b + 1] = load_samp(b + 1, nc.scalar)
                        combs[b + 1] = combp.tile([128, CCH, OCH, QH],
                                                  BF16, name="comb")
                        combine_sample(b + 1, combs[b + 1])

        if iters == 1:
            body()
        else:
            # On-device repeat loop — used only for slope-based HW timing.
            with tc.For_i(0, iters, 1, hint_engines=(mybir.EngineType.PE,)):
                body()

    _dedup_ldweights(nc)
    nc.compile()
    return nc


def prep_inputs(input, attention, weights, bias):
    """Host-side shard + layout prep. Returns per-core in_maps."""
    import ml_dtypes
    input = np.asarray(input, dtype=np.float32)
    attention = np.asarray(attention, dtype=np.float32)
    weights = np.asarray(weights, dtype=np.float32)
    bias = np.asarray(bias, dtype=np.float32)

    xp = np.zeros((B, CCH, 128, PW, PW), ml_dtypes.bfloat16)
    xp[:, :, :, 1:H + 1, 1:W + 1] = input.reshape(B, CCH, 128, H, W)
    xp = np.ascontiguousarray(xp.transpose(0, 2, 1, 3, 4))  # [B,128,CCH,PW,PW]

    # weights [E, O, C, ky, kx] -> bank[e, p(c_lo), c_chunk, o_chunk, (d,o_lo)]
    wt = weights.transpose(0, 2, 3, 4, 1)                    # [E, C, ky, kx, O]
    wt = wt.reshape(E, CCH, 128, DYX, OCH, 128)              # [E,k,p,d,j,o]
    bank = np.ascontiguousarray(wt.transpose(0, 2, 1, 4, 3, 5)
                                ).reshape(E, 128, CCH, OCH, QH)
    bank = bank.astype(ml_dtypes.bfloat16)

    bias_tp = np.ascontiguousarray(
        bias.T.reshape(OCH, 128, E).transpose(1, 0, 2))      # [128, OCH, E]

    in_maps = []
    for m in range(N_CORES):
        sl = slice(m * B_LOC, (m + 1) * B_LOC)
        att_m = np.ascontiguousarray(
            np.broadcast_to(attention[sl].reshape(1, B_LOC * E),
                            (128, B_LOC * E)))
        in_maps.append({
            "x": np.ascontiguousarray(xp[sl]),
            "bank": bank,
            "att": att_m,
            "bias_t": bias_tp,
        })
    return in_maps


def gather_output(results):
    """Per-core [B_LOC, 128, OCH, H, W] -> full [B, O, H, W]."""
    outs = []
    for m in range(N_CORES):
        o = results[m]["out"]  # [B_LOC, 128, OCH, H, W]
        outs.append(o.transpose(0, 2, 1, 3, 4).reshape(B_LOC, O, H, W))
    return np.concatenate(outs, axis=0)


_NC_CACHE = {}


def _get_nc():
    if "nc" not in _NC_CACHE:
        _NC_CACHE["nc"] = build()
    return _NC_CACHE["nc"]


def kernel(input, attention, weights, bias):
    nc = _get_nc()
    in_maps = prep_inputs(input, attention, weights, bias)
    res = run_bass_kernel_spmd(nc, in_maps, list(range(N_CORES)))
    return gather_output(res.results)


# revision 13
# speedup vs baseline: 2.9434x; 2.9434x over previous
"""Dynamic-expert-conv kernel for Trainium2 (8 NeuronCores, SPMD data-parallel).

Problem: per-sample expert-mixed 3x3 conv:
    w[b] = sum_e attention[b,e] * weights[e]     # [O, C, 3, 3]
    out[b] = conv2d(input[b], w[b], pad=1) + bias_mix[b][:, None, None]

Strategy (per core, 4 samples):
  - Expert weight bank resident in SBUF as bf16 (halves HBM+SBUF cost).
  - Per-sample combined weights built as a bf16 product/add TREE split
    across DVE and Pool: products via tensor_scalar_mul (4x DVE mode,
    ~0.31 ns/col) and pairwise tensor_tensor adds (2x mode, ~0.57),
    instead of the 1x scalar_tensor_tensor chain (~1.08). Combine drops
    from ~41us to ~10us/engine per sample, so it never gates the PE.
  - Conv as accumulating bf16 PE matmuls with the STATIONARY operand
    maximally reused: ONE sweep per (sample, o-chunk) over all 7 y-tile
    PSUM banks, (c-chunk, offset) outer / y-tiles inner. 18 stationary
    loads per sweep feed 7x448 moving columns each (144 LDWEIGHTS total
    after dedup, vs 288 for split groups).
  - PSUM evacuation (bias fused) round-robins over ScalarE (activation),
    DVE and Pool (tensor_scalar_add) so banks free at ~3x one engine's
    rate; the 8th PSUM bank is a rotation spare so the next sweep's
    first matmul never waits.
  - DMA rings: sync = att/bias/bank + all output stores; scalar = half
    the bank + per-sample input loads (issued between evac groups);
    gpsimd = sample-0 input only (Pool is combining the rest of the
    time). Input is bf16, host pre-padded to 58x58.
"""
import numpy as np

import concourse.bass as bass
import concourse.tile as tile
from concourse import bacc, mybir
from concourse.bass import ts
from concourse.bass_utils import run_bass_kernel_spmd
from contextlib import ExitStack

F32 = mybir.dt.float32
BF16 = mybir.dt.bfloat16
ADD = mybir.AluOpType.add
MULT = mybir.AluOpType.mult

import os
# tree_pool: product/add tree split DVE+Pool; tree_dve: tree all on DVE;
# stt: baseline-style scalar_tensor_tensor chain on DVE (fp32 accum).
COMBINE_MODE = os.environ.get("COMBINE_MODE", "tree_pool")

B, C, O, H, W, KK, E = 32, 256, 256, 56, 56, 3, 8
N_CORES = 8
B_LOC = B // N_CORES          # 4 samples per core
PW = H + 2                    # 58 padded
CCH = C // 128                # 2
OCH = O // 128                # 2
YT = 8                        # output rows per tile
NT = H // YT                  # 7 y-tiles
DYX = KK * KK                 # 9
QH = DYX * 128                # 1152 combined-weight cols per (c-chunk, o-chunk)


def _dedup_ldweights(nc):
    """Drop redundant PE weight reloads.

    Tile lowering splits every bf16 matmul into Ldweights+Matmult; the HW
    pays a serial ~54ns per LDWEIGHTS and does not skip reloads of the
    already-loaded stationary. conv_sweep orders matmuls so consecutive
    ones share the stationary, so any Ldweights whose access pattern
    matches the previous Ldweights in the same block's PE stream (with no
    waits/updates of its own) is a no-op and can be deleted: the weights
    are still in the array, and its (empty) sync carries nothing."""
    removed = 0
    for blk in nc.m.functions[0].blocks:
        last_key = None
        keep = []
        for inst in blk.instructions:
            if isinstance(inst, mybir.InstLdweights):
                si = inst.sync_info
                clean = si is None or (not si.on_wait and not si.on_update)
                key = (str(inst.ins[0]), str(inst.tile_position),
                       str(inst.perf_mode), str(inst.is_transpose))
                if clean and key == last_key:
                    removed += 1
                    continue
                last_key = key
            keep.append(inst)
        if removed:
            blk.instructions = keep
    return removed


def build(iters: int = 1):
    nc = bacc.Bacc("TRN2", target_bir_lowering=False, debug=False,
                   num_devices=N_CORES)
    x = nc.dram_tensor("x", [B_LOC, 128, CCH, PW, PW], BF16,
                       kind="ExternalInput").ap()
    bank = nc.dram_tensor("bank", [E, 128, CCH, OCH, QH], BF16,
                          kind="ExternalInput").ap()
    att = nc.dram_tensor("att", [128, B_LOC * E], F32,
                         kind="ExternalInput").ap()
    bias_t = nc.dram_tensor("bias_t", [128, OCH, E], F32,
                            kind="ExternalInput").ap()
    out = nc.dram_tensor("out", [B_LOC, 128, OCH, H, W], F32,
                         kind="ExternalOutput").ap()

    with ExitStack() as ctx:
        tc = ctx.enter_context(tile.TileContext(nc))
        const = ctx.enter_context(tc.tile_pool(name="const", bufs=1))
        bankp = ctx.enter_context(tc.tile_pool(name="bankp", bufs=1))
        combp = ctx.enter_context(tc.tile_pool(name="combp", bufs=2))
        tmpp = ctx.enter_context(tc.tile_pool(name="tmpp", bufs=16))
        sampp = ctx.enter_context(tc.tile_pool(name="sampp", bufs=2))
        stagep = ctx.enter_context(tc.tile_pool(name="stagep", bufs=8))
        psump = ctx.enter_context(tc.tile_pool(name="psump", bufs=8,
                                               space="PSUM"))

        # att/bias first on the sync ring (tiny); the scalar ring opens
        # with the act-table load for the evacuation activations.
        att_sb = const.tile([128, B_LOC * E], F32)
        nc.sync.dma_start(att_sb[:], att[:])
        bias_sb = const.tile([128, OCH, E], F32)
        nc.sync.dma_start(bias_sb[:], bias_t[:])

        bank_sb = bankp.tile([128, E, CCH, OCH, QH], BF16)
        # Stream the bank in the (j,k) order combine consumes it, with
        # experts split across the sync and scalar rings so each quarter
        # lands ~2x sooner. The j=0 quarters (PE-start critical path) are
        # further split into the 3 column slices the chunked prologue
        # combine consumes, slice-major, so chunk 0's inputs all land
        # ~2.5us earlier than whole pieces would.
        for (j, k) in ((0, 0), (0, 1), (1, 0), (1, 1)):
            slices = ((0, 128), (128, 640), (640, QH)) if j == 0 \
                else ((0, QH),)
            for lo, hi in slices:
                for e in range(E):
                    ring = nc.sync if e % 2 == 0 else nc.scalar
                    ring.dma_start(bank_sb[:, e, k, j, lo:hi],
                                   bank[e][:, k, j, lo:hi])

        bias_comb = const.tile([128, B_LOC, OCH], F32)
        bias_junk = const.tile([128, E], F32)

        def combine_bias(b):
            # bias_comb[p, b, j] = sum_e bias_t[p, j, e] * att[p, b*E+e]
            for j in range(OCH):
                nc.vector.scalar_tensor_tensor(
                    bias_junk[:], bias_sb[:, j, :], 1.0,
                    att_sb[:, b * E:(b + 1) * E],
                    op0=MULT, op1=MULT,
                    accum_out=bias_comb[:, b, j:j + 1])

        def combine_quarter(b, cb, j, k, chunks=1, pool_share=True):
            """cb[:,k,j,:] = sum_e att[b,e]*bank[:,e,k,j,:], all in bf16.

            Product/add tree: products via tensor_scalar_mul (DVE 4x
            mode), pairwise adds via tensor_tensor (DVE 2x). With
            pool_share, experts 6,7 and the upper merges run on Pool
            (~balanced: DVE 6 mul + 4 add, Pool 2 mul + 3 add).
            `chunks` splits the columns so the first slice is ready for
            the PE sooner (prologue only)."""
            a = [att_sb[:, b * E + e:b * E + e + 1] for e in range(E)]
            if COMBINE_MODE == "stt":
                # Baseline-style fp32 stt chain (accumulate in a fp32
                # scratch, cast to bf16).
                cf = tmpp.tile([128, QH], F32, name="tmpf")
                dst = cf[:]
                srcs = [bank_sb[:, e, k, j, :] for e in range(E)]
                nc.vector.tensor_scalar_mul(dst, srcs[0], a[0])
                for e in range(1, E):
                    nc.vector.scalar_tensor_tensor(
                        dst, srcs[e], a[e], dst, op0=MULT, op1=ADD)
                nc.vector.tensor_scalar_mul(cb[:, k, j, :], dst, 1.0)
                return
            t = [tmpp.tile([128, QH], BF16, name="tmp") for _ in range(8)]
            V = nc.vector
            P = nc.gpsimd if (pool_share and COMBINE_MODE == "tree_pool") \
                else nc.vector
            if chunks == 1:
                bounds = ((0, QH),)
            else:
                # Tiny first chunk so the PE's first matmul (which only
                # needs cols 0:128) starts as early as possible.
                bounds = ((0, 128), (128, 640), (640, QH))
            for lo_c, hi_c in bounds:
                s = slice(lo_c, hi_c)
                bk = [bank_sb[:, e, k, j, s] for e in range(E)]
                V.tensor_scalar_mul(t[0][:, s], bk[0], a[0])
                V.tensor_scalar_mul(t[1][:, s], bk[1], a[1])
                V.tensor_tensor(t[0][:, s], t[0][:, s], t[1][:, s], op=ADD)
                V.tensor_scalar_mul(t[2][:, s], bk[2], a[2])
                V.tensor_scalar_mul(t[3][:, s], bk[3], a[3])
                V.tensor_tensor(t[2][:, s], t[2][:, s], t[3][:, s], op=ADD)
                V.tensor_tensor(t[0][:, s], t[0][:, s], t[2][:, s], op=ADD)
                V.tensor_scalar_mul(t[4][:, s], bk[4], a[4])
                V.tensor_scalar_mul(t[5][:, s], bk[5], a[5])
                V.tensor_tensor(t[4][:, s], t[4][:, s], t[5][:, s], op=ADD)
                P.tensor_scalar_mul(t[6][:, s], bk[6], a[6])
                P.tensor_scalar_mul(t[7][:, s], bk[7], a[7])
                P.tensor_tensor(t[6][:, s], t[6][:, s], t[7][:, s], op=ADD)
                P.tensor_tensor(t[4][:, s], t[4][:, s], t[6][:, s], op=ADD)
                P.tensor_tensor(cb[:, k, j, s], t[0][:, s], t[4][:, s],
                                op=ADD)

        def combine_sample(b, cb, prologue=False):
            for j in range(OCH):
                for k in range(CCH):
                    if prologue and j == 0:
                        # Fine-grained, DVE-only (Pool busy with the
                        # sample-0 input DMA until ~5.5us).
                        combine_quarter(b, cb, j, k, chunks=3,
                                        pool_share=False)
                    else:
                        combine_quarter(b, cb, j, k)
                if j == 0:
                    combine_bias(b)

        def load_samp(b, ring):
            samp = sampp.tile([128, CCH, PW, PW], BF16, name="samp")
            ring.dma_start(samp[:], x[b][:])
            return samp

        def conv_sweep(j, comb, samp):
            """One stationary-load sweep per (sample, o-chunk): (k,d)
            outer so consecutive matmuls share one combined-weight tile
            (_dedup_ldweights drops their redundant reloads) and stream
            7x448 moving columns per load; all 7 y-tiles accumulate in
            parallel PSUM banks."""
            psums = [psump.tile([128, YT, W], F32, name="psum")
                     for _ in range(NT)]
            for k in range(CCH):
                for d in range(DYX):
                    dy, dx = d // KK, d % KK
                    lhsT = comb[:, k, j, d * 128:(d + 1) * 128]
                    first = (k == 0 and d == 0)
                    last = (k == CCH - 1 and d == DYX - 1)
                    for i in range(NT):
                        r0 = i * YT + dy
                        rhs = samp[:, k, r0:r0 + YT, dx:dx + W]
                        nc.tensor.matmul(psums[i][:], lhsT, rhs,
                                         start=first, stop=last)
            return psums

        def evac_sweep(b, j, psums, last=False):
            """PSUM -> SBUF fp32 with mixed bias fused, round-robin over
            ScalarE / DVE / Pool so banks free ~3x faster; store on the
            sync ring (the final sweep spreads stores over all 3 rings
            to cut the drain tail ~3x)."""
            stages = []
            for i in range(NT):
                stage = stagep.tile([128, YT, W], F32, name="stage")
                # Only ScalarE and DVE may read PSUM (GPSIMD is rejected
                # by the BIR verifier); alternate them so banks free at
                # 2x one engine's rate.
                if i % 2 == 0:
                    nc.scalar.activation(
                        stage[:], psums[i][:],
                        mybir.ActivationFunctionType.Identity,
                        bias=bias_comb[:, b, j:j + 1], scale=1.0)
                else:
                    nc.vector.tensor_scalar_add(stage[:], psums[i][:],
                                                bias_comb[:, b, j:j + 1])
                stages.append(stage)
            rings = ((nc.sync, nc.scalar, nc.gpsimd) if last
                     else (nc.sync,))
            for i in range(NT):
                rings[i % len(rings)].dma_start(
                    out[b][:, j:j + 1, ts(i, YT), :], stages[i][:])

        def body():
            samps = {0: load_samp(0, nc.gpsimd)}
            combs = {0: combp.tile([128, CCH, OCH, QH], BF16, name="comb")}
            combine_sample(0, combs[0], prologue=True)
            for b in range(B_LOC):
                for j in range(OCH):
                    psums = conv_sweep(j, combs[b], samps[b])
                    evac_sweep(b, j, psums,
                               last=(b == B_LOC - 1 and j == OCH - 1))
                    if j == 0 and b + 1 < B_LOC:
                        # Issue next sample's input + combine after the
                        # evac ops so PSUM turnaround never queues behind
                        # them; they execute during this sample's j=1.
                        samps[b + 1] = load_samp(b + 1, nc.scalar)
                        combs[b + 1] = combp.tile([128, CCH, OCH, QH],
                                                  BF16, name="comb")
                        combine_sample(b + 1, combs[b + 1])

        if iters == 1:
            body()
        else:
            # On-device repeat loop — used only for slope-based HW timing.
            with tc.For_i(0, iters, 1, hint_engines=(mybir.EngineType.PE,)):
                body()

    _dedup_ldweights(nc)
    nc.compile()
    return nc


def prep_inputs(input, attention, weights, bias):
    """Host-side shard + layout prep. Returns per-core in_maps."""
    import ml_dtypes
    input = np.asarray(input, dtype=np.float32)
    attention = np.asarray(attention, dtype=np.float32)
    weights = np.asarray(weights, dtype=np.float32)
    bias = np.asarray(bias, dtype=np.float32)

    xp = np.zeros((B, CCH, 128, PW, PW), ml_dtypes.bfloat16)
    xp[:, :, :, 1:H + 1, 1:W + 1] = input.reshape(B, CCH, 128, H, W)
    xp = np.ascontiguousarray(xp.transpose(0, 2, 1, 3, 4))  # [B,128,CCH,PW,PW]

    # weights [E, O, C, ky, kx] -> bank[e, p(c_lo), c_chunk, o_chunk, (d,o_lo)]
    wt = weights.transpose(0, 2, 3, 4, 1)                    # [E, C, ky, kx, O]
    wt = wt.reshape(E, CCH, 128, DYX, OCH, 128)              # [E,k,p,d,j,o]
    bank = np.ascontiguousarray(wt.transpose(0, 2, 1, 4, 3, 5)
                                ).reshape(E, 128, CCH, OCH, QH)
    bank = bank.astype(ml_dtypes.bfloat16)

    bias_tp = np.ascontiguousarray(
        bias.T.reshape(OCH, 128, E).transpose(1, 0, 2))      # [128, OCH, E]

    in_maps = []
    for m in range(N_CORES):
        sl = slice(m * B_LOC, (m + 1) * B_LOC)
        att_m = np.ascontiguousarray(
            np.broadcast_to(attention[sl].reshape(1, B_LOC * E),
                            (128, B_LOC * E)))
        in_maps.append({
            "x": np.ascontiguousarray(xp[sl]),
            "bank": bank,
            "att": att_m,
            "bias_t": bias_tp,
        })
    return in_maps


def gather_output(results):
    """Per-core [B_LOC, 128, OCH, H, W] -> full [B, O, H, W]."""
    outs = []
    for m in range(N_CORES):
        o = results[m]["out"]  # [B_LOC, 128, OCH, H, W]
        outs.append(o.transpose(0, 2, 1, 3, 4).reshape(B_LOC, O, H, W))
    return np.concatenate(outs, axis=0)


_NC_CACHE = {}


def _get_nc():
    if "nc" not in _NC_CACHE:
        _NC_CACHE["nc"] = build()
    return _NC_CACHE["nc"]


def kernel(input, attention, weights, bias):
    nc = _get_nc()
    in_maps = prep_inputs(input, attention, weights, bias)
    res = run_bass_kernel_spmd(nc, in_maps, list(range(N_CORES)))
    return gather_output(res.results)


# revision 14
# speedup vs baseline: 3.4000x; 1.1551x over previous
"""Dynamic-expert-conv kernel for Trainium2 (8 NeuronCores, SPMD data-parallel).

Problem: per-sample expert-mixed 3x3 conv:
    w[b] = sum_e attention[b,e] * weights[e]     # [O, C, 3, 3]
    out[b] = conv2d(input[b], w[b], pad=1) + bias_mix[b][:, None, None]

Strategy (per core, 4 samples), from HW microbenching (PE-only floor for
this matmul structure is ~197us; PSUM evacuation costs ~+20us on top
regardless of scheme; gpsimd tensor ops are ~10x slower than modeled and
must not be used for compute):
  - Expert weight bank resident in SBUF as bf16.
  - Per-sample combined weights built on DVE only, as a bf16 product/add
    tree: products via tensor_scalar_mul (4x DVE mode) and pairwise adds
    via tensor_tensor (2x mode) -- ~30us/sample, leaving DVE ~40% idle so
    it never gates the PE (the old fp32 stt chain was ~41us/sample).
  - Conv as accumulating bf16 PE matmuls, stationary reused across a
    group of y-tile PSUM banks: per (sample, o-chunk) two groups of 4+3
    banks, (c-chunk, offset) outer / y-tiles inner. Groups double-buffer
    PSUM: group A's banks evacuate while group B computes, so the PE
    never waits on a bank (single 7-bank sweeps measured slower on HW:
    the boundary evac drain stalls the PE and resets its p-state ramp).
  - PSUM evacuation (bias fused) entirely on ScalarE activations; output
    stores on the sync ring (the final groups spread stores over sync/
    scalar/gpsimd to cut the drain tail).
  - DMA rings: sync = att/bias/bank(even experts) + output stores;
    scalar = bank(odd experts) behind the act-table load; gpsimd =
    input loads ONLY (so evacuations never queue behind a 5us input
    DMA). Input is bf16, host pre-padded to 58x58.
  - Startup: the j=0 bank quarters stream in 3 column slices and the
    prologue combine is chunked (128/512/512 cols) so the PE's first
    matmul issues ~6us in instead of ~14us.
"""
import numpy as np

import concourse.bass as bass
import concourse.tile as tile
from concourse import bacc, mybir
from concourse.bass import ts
from concourse.bass_utils import run_bass_kernel_spmd
from contextlib import ExitStack

F32 = mybir.dt.float32
BF16 = mybir.dt.bfloat16
ADD = mybir.AluOpType.add
MULT = mybir.AluOpType.mult

import os
# tree: bf16 product/add tree on DVE; stt: fp32 scalar_tensor_tensor chain.
COMBINE_MODE = os.environ.get("COMBINE_MODE", "tree")

B, C, O, H, W, KK, E = 32, 256, 256, 56, 56, 3, 8
N_CORES = 8
B_LOC = B // N_CORES          # 4 samples per core
PW = H + 2                    # 58 padded
CCH = C // 128                # 2
OCH = O // 128                # 2
YT = 8                        # output rows per tile
NT = H // YT                  # 7 y-tiles
DYX = KK * KK                 # 9
QH = DYX * 128                # 1152 combined-weight cols per (c-chunk, o-chunk)

# y-tile groups sharing one stationary-load sweep (4+3 PSUM banks,
# double-buffered against each other)
GROUPS = [(0, 4), (4, 3)]


def _dedup_ldweights(nc):
    """Drop redundant PE weight reloads.

    Tile lowering splits every bf16 matmul into Ldweights+Matmult; the HW
    pays a serial ~54ns per LDWEIGHTS and does not skip reloads of the
    already-loaded stationary. conv_group orders matmuls so consecutive
    ones share the stationary, so any Ldweights whose access pattern
    matches the previous Ldweights in the same block's PE stream (with no
    waits/updates of its own) is a no-op and can be deleted: the weights
    are still in the array, and its (empty) sync carries nothing."""
    removed = 0
    for blk in nc.m.functions[0].blocks:
        last_key = None
        keep = []
        for inst in blk.instructions:
            if isinstance(inst, mybir.InstLdweights):
                si = inst.sync_info
                clean = si is None or (not si.on_wait and not si.on_update)
                key = (str(inst.ins[0]), str(inst.tile_position),
                       str(inst.perf_mode), str(inst.is_transpose))
                if clean and key == last_key:
                    removed += 1
                    continue
                last_key = key
            keep.append(inst)
        if removed:
            blk.instructions = keep
    return removed


def build(iters: int = 1):
    nc = bacc.Bacc("TRN2", target_bir_lowering=False, debug=False,
                   num_devices=N_CORES)
    x = nc.dram_tensor("x", [B_LOC, 128, CCH, PW, PW], BF16,
                       kind="ExternalInput").ap()
    bank = nc.dram_tensor("bank", [E, 128, CCH, OCH, QH], BF16,
                          kind="ExternalInput").ap()
    att = nc.dram_tensor("att", [128, B_LOC * E], F32,
                         kind="ExternalInput").ap()
    bias_t = nc.dram_tensor("bias_t", [128, OCH, E], F32,
                            kind="ExternalInput").ap()
    out = nc.dram_tensor("out", [B_LOC, 128, OCH, H, W], F32,
                         kind="ExternalOutput").ap()

    with ExitStack() as ctx:
        tc = ctx.enter_context(tile.TileContext(nc))
        const = ctx.enter_context(tc.tile_pool(name="const", bufs=1))
        bankp = ctx.enter_context(tc.tile_pool(name="bankp", bufs=1))
        combp = ctx.enter_context(tc.tile_pool(name="combp", bufs=2))
        tmpp = ctx.enter_context(tc.tile_pool(name="tmpp", bufs=16))
        sampp = ctx.enter_context(tc.tile_pool(name="sampp", bufs=2))
        stagep = ctx.enter_context(tc.tile_pool(name="stagep", bufs=8))
        psump = ctx.enter_context(tc.tile_pool(name="psump", bufs=8,
                                               space="PSUM"))

        # att/bias first on the sync ring (tiny); the scalar ring opens
        # with the act-table load for the evacuation activations.
        att_sb = const.tile([128, B_LOC * E], F32)
        nc.sync.dma_start(att_sb[:], att[:])
        bias_sb = const.tile([128, OCH, E], F32)
        nc.sync.dma_start(bias_sb[:], bias_t[:])

        bank_sb = bankp.tile([128, E, CCH, OCH, QH], BF16)
        # Stream the bank in the (j,k) order combine consumes it, with
        # experts split across the sync and scalar rings so each quarter
        # lands ~2x sooner. The j=0 quarters (PE-start critical path) are
        # further split into the 3 column slices the chunked prologue
        # combine consumes, slice-major, so chunk 0's inputs all land
        # ~2.5us earlier than whole pieces would.
        for (j, k) in ((0, 0), (0, 1), (1, 0), (1, 1)):
            slices = ((0, 128), (128, 640), (640, QH)) if j == 0 \
                else ((0, QH),)
            for lo, hi in slices:
                for e in range(E):
                    ring = nc.sync if e % 2 == 0 else nc.scalar
                    ring.dma_start(bank_sb[:, e, k, j, lo:hi],
                                   bank[e][:, k, j, lo:hi])

        bias_comb = const.tile([128, B_LOC, OCH], F32)
        bias_junk = const.tile([128, E], F32)

        def combine_bias(b):
            # bias_comb[p, b, j] = sum_e bias_t[p, j, e] * att[p, b*E+e]
            for j in range(OCH):
                nc.vector.scalar_tensor_tensor(
                    bias_junk[:], bias_sb[:, j, :], 1.0,
                    att_sb[:, b * E:(b + 1) * E],
                    op0=MULT, op1=MULT,
                    accum_out=bias_comb[:, b, j:j + 1])

        def combine_quarter(b, cb, j, k, chunks=1):
            """cb[:,k,j,:] = sum_e att[b,e]*bank[:,e,k,j,:], in bf16 on
            DVE: products via tensor_scalar_mul (4x mode), pairwise adds
            via tensor_tensor (2x). `chunks` splits the columns so the
            first slice is ready for the PE sooner (prologue only)."""
            a = [att_sb[:, b * E + e:b * E + e + 1] for e in range(E)]
            V = nc.vector
            if COMBINE_MODE == "stt":
                cf = tmpp.tile([128, QH], F32, name="tmpf")
                dst = cf[:]
                srcs = [bank_sb[:, e, k, j, :] for e in range(E)]
                V.tensor_scalar_mul(dst, srcs[0], a[0])
                for e in range(1, E):
                    V.scalar_tensor_tensor(
                        dst, srcs[e], a[e], dst, op0=MULT, op1=ADD)
                V.tensor_scalar_mul(cb[:, k, j, :], dst, 1.0)
                return
            t = [tmpp.tile([128, QH], BF16, name="tmp") for _ in range(8)]
            if chunks == 1:
                bounds = ((0, QH),)
            else:
                # Tiny first chunk so the PE's first matmul (which only
                # needs cols 0:128) starts as early as possible.
                bounds = ((0, 128), (128, 640), (640, QH))
            for lo_c, hi_c in bounds:
                s = slice(lo_c, hi_c)
                bk = [bank_sb[:, e, k, j, s] for e in range(E)]
                for e in range(E):
                    V.tensor_scalar_mul(t[e][:, s], bk[e], a[e])
                V.tensor_tensor(t[0][:, s], t[0][:, s], t[1][:, s], op=ADD)
                V.tensor_tensor(t[2][:, s], t[2][:, s], t[3][:, s], op=ADD)
                V.tensor_tensor(t[4][:, s], t[4][:, s], t[5][:, s], op=ADD)
                V.tensor_tensor(t[6][:, s], t[6][:, s], t[7][:, s], op=ADD)
                V.tensor_tensor(t[0][:, s], t[0][:, s], t[2][:, s], op=ADD)
                V.tensor_tensor(t[4][:, s], t[4][:, s], t[6][:, s], op=ADD)
                V.tensor_tensor(cb[:, k, j, s], t[0][:, s], t[4][:, s],
                                op=ADD)

        def combine_sample(b, cb, prologue=False):
            for j in range(OCH):
                for k in range(CCH):
                    chunks = 3 if (prologue and j == 0) else 1
                    combine_quarter(b, cb, j, k, chunks=chunks)
                if j == 0:
                    combine_bias(b)

        def load_samp(b):
            # Dedicated gpsimd ring: an input DMA must never delay the
            # evacuation activations (scalar ring) or output stores
            # (sync ring).
            samp = sampp.tile([128, CCH, PW, PW], BF16, name="samp")
            nc.gpsimd.dma_start(samp[:], x[b][:])
            return samp

        def conv_group(j, t0, tn, comb, samp):
            """One stationary-load sweep over y-tiles [t0, t0+tn):
            (k,d) outer so consecutive matmuls share one combined-weight
            tile (_dedup_ldweights drops their redundant reloads) and
            stream tn x 448 moving columns per load; y-tiles accumulate
            in parallel PSUM banks."""
            psums = [psump.tile([128, YT, W], F32, name="psum")
                     for _ in range(tn)]
            for k in range(CCH):
                for d in range(DYX):
                    dy, dx = d // KK, d % KK
                    lhsT = comb[:, k, j, d * 128:(d + 1) * 128]
                    first = (k == 0 and d == 0)
                    last = (k == CCH - 1 and d == DYX - 1)
                    for i in range(tn):
                        r0 = (t0 + i) * YT + dy
                        rhs = samp[:, k, r0:r0 + YT, dx:dx + W]
                        nc.tensor.matmul(psums[i][:], lhsT, rhs,
                                         start=first, stop=last)
            return psums

        def evac_group(b, j, t0, tn, psums, last=False):
            """PSUM -> SBUF fp32 with mixed bias fused, on ScalarE; the
            final groups spread stores over all 3 rings to cut the
            drain tail."""
            rings = ((nc.sync, nc.scalar, nc.gpsimd) if last
                     else (nc.sync,))
            for i in range(tn):
                stage = stagep.tile([128, YT, W], F32, name="stage")
                nc.scalar.activation(
                    stage[:], psums[i][:],
                    mybir.ActivationFunctionType.Identity,
                    bias=bias_comb[:, b, j:j + 1], scale=1.0)
                rings[i % len(rings)].dma_start(
                    out[b][:, j:j + 1, ts(t0 + i, YT), :], stage[:])

        def body():
            samps = {0: load_samp(0)}
            combs = {0: combp.tile([128, CCH, OCH, QH], BF16, name="comb")}
            combine_sample(0, combs[0], prologue=True)
            for b in range(B_LOC):
                for j in range(OCH):
                    last_j = (b == B_LOC - 1 and j == OCH - 1)
                    for t0, tn in GROUPS:
                        psums = conv_group(j, t0, tn, combs[b], samps[b])
                        evac_group(b, j, t0, tn, psums, last=last_j)
                    if j == 0 and b + 1 < B_LOC:
                        samps[b + 1] = load_samp(b + 1)
                        combs[b + 1] = combp.tile([128, CCH, OCH, QH],
                                                  BF16, name="comb")
                        combine_sample(b + 1, combs[b + 1])

        if iters == 1:
            body()
        else:
            # On-device repeat loop — used only for slope-based HW timing.
            with tc.For_i(0, iters, 1, hint_engines=(mybir.EngineType.PE,)):
                body()

    _dedup_ldweights(nc)
    nc.compile()
    return nc


def prep_inputs(input, attention, weights, bias):
    """Host-side shard + layout prep. Returns per-core in_maps."""
    import ml_dtypes
    input = np.asarray(input, dtype=np.float32)
    attention = np.asarray(attention, dtype=np.float32)
    weights = np.asarray(weights, dtype=np.float32)
    bias = np.asarray(bias, dtype=np.float32)

    xp = np.zeros((B, CCH, 128, PW, PW), ml_dtypes.bfloat16)
    xp[:, :, :, 1:H + 1, 1:W + 1] = input.reshape(B, CCH, 128, H, W)
    xp = np.ascontiguousarray(xp.transpose(0, 2, 1, 3, 4))  # [B,128,CCH,PW,PW]

    # weights [E, O, C, ky, kx] -> bank[e, p(c_lo), c_chunk, o_chunk, (d,o_lo)]
    wt = weights.transpose(0, 2, 3, 4, 1)                    # [E, C, ky, kx, O]
    wt = wt.reshape(E, CCH, 128, DYX, OCH, 128)              # [E,k,p,d,j,o]
    bank = np.ascontiguousarray(wt.transpose(0, 2, 1, 4, 3, 5)
                                ).reshape(E, 128, CCH, OCH, QH)
    bank = bank.astype(ml_dtypes.bfloat16)

    bias_tp = np.ascontiguousarray(
        bias.T.reshape(OCH, 128, E).transpose(1, 0, 2))      # [128, OCH, E]

    in_maps = []
    for m in range(N_CORES):
        sl = slice(m * B_LOC, (m + 1) * B_LOC)
        att_m = np.ascontiguousarray(
            np.broadcast_to(attention[sl].reshape(1, B_LOC * E),
                            (128, B_LOC * E)))
        in_maps.append({
            "x": np.ascontiguousarray(xp[sl]),
            "bank": bank,
            "att": att_m,
            "bias_t": bias_tp,
        })
    return in_maps


def gather_output(results):
    """Per-core [B_LOC, 128, OCH, H, W] -> full [B, O, H, W]."""
    outs = []
    for m in range(N_CORES):
        o = results[m]["out"]  # [B_LOC, 128, OCH, H, W]
        outs.append(o.transpose(0, 2, 1, 3, 4).reshape(B_LOC, O, H, W))
    return np.concatenate(outs, axis=0)


_NC_CACHE = {}


def _get_nc():
    if "nc" not in _NC_CACHE:
        _NC_CACHE["nc"] = build()
    return _NC_CACHE["nc"]


def kernel(input, attention, weights, bias):
    nc = _get_nc()
    in_maps = prep_inputs(input, attention, weights, bias)
    res = run_bass_kernel_spmd(nc, in_maps, list(range(N_CORES)))
    return gather_output(res.results)
